# revision 22
# baseline (speedup 1.0000x reference)
"""Multi-head attention (B=2, S=2048, D=1024, 16 heads x 64) on 8 TRN2 cores.

Sharding: tensor-parallel over heads. Core c owns heads {2c, 2c+1} =
rows [128c, 128c+128) of Wq/Wk/Wv, computes its (B, S, 128) slice of the
context, host concatenates along the feature axis. No collectives.

Per-core pipeline (matmul operands bf16, f32 PSUM accumulation):
  x, W: f32 HWDGE load -> DVE cast to bf16 -> PE transpose (1 cyc/row,
  4 chunks packed per PSUM bank) -> DVE copy to xT/wT.
  qT/kT/vT projections (+bias per-partition). v re-transposed to [t, w]
  on PE. mask -> em[t] = exp(-1e4*(1-mask[t])) folded into V rows
  (exact: exp(a+b) = exp(a)exp(b)); V carries an extra em column so the
  PV matmul also produces the softmax denominator Z.
  scoresT[t,s] = k[t].q[s] per 128-key chunk -> ACT exp (scale=1/8) from
  PSUM -> PV accumulate (65 x 512) -> PE transpose -> scale by 1/Z -> out
  (output DMA on GPSIMD/SWDGE to keep the HWDGE queues free).
"""

import sys

if "/opt/trn_rl_repo" not in sys.path:
    sys.path.insert(0, "/opt/trn_rl_repo")

import numpy as np

B = 2
S = 2048
D = 1024
NCORES = 8
WC = 128          # per-core projection width (2 heads x 64)
HEADS = 2         # heads per core
W = 64            # head dim
KC = D // 128     # contraction chunks (8)
SC = S // 128     # 128-row chunks of S (16)
SEG = 512         # matmul moving-dim segment
NSEG = S // SEG   # 4
SBLK = 1024       # attention s-block (2 segments)
NBLK = S // SBLK  # 2


def _build():
    import concourse.bass as bass
    import concourse.tile as tile
    from concourse import bacc, mybir
    from concourse.masks import make_identity

    f32 = mybir.dt.float32
    bf16 = mybir.dt.bfloat16
    EXP = mybir.ActivationFunctionType.Exp

    nc = bacc.Bacc("TRN2", target_bir_lowering=False, debug=False)

    x_d = nc.dram_tensor("hidden_states", [B, S, D], f32, kind="ExternalInput")
    m_d = nc.dram_tensor("attn_mask", [B, S], f32, kind="ExternalInput")
    wq_d = nc.dram_tensor("wq", [WC, D], f32, kind="ExternalInput")
    wk_d = nc.dram_tensor("wk", [WC, D], f32, kind="ExternalInput")
    wv_d = nc.dram_tensor("wv", [WC, D], f32, kind="ExternalInput")
    bq_d = nc.dram_tensor("bq", [WC], f32, kind="ExternalInput")
    bk_d = nc.dram_tensor("bk", [WC], f32, kind="ExternalInput")
    bv_d = nc.dram_tensor("bv", [WC], f32, kind="ExternalInput")
    o_d = nc.dram_tensor("out", [B, S, WC], f32, kind="ExternalOutput")

    with tile.TileContext(nc) as tc:
        consts = tc.alloc_tile_pool(name="consts", bufs=1)
        xp = tc.alloc_tile_pool(name="xp", bufs=4)
        xbp = tc.alloc_tile_pool(name="xbp", bufs=4)
        xtp = tc.alloc_tile_pool(name="xtp", bufs=2)
        qkp = tc.alloc_tile_pool(name="qkp", bufs=2)
        vp = tc.alloc_tile_pool(name="vp", bufs=2)
        etp = tc.alloc_tile_pool(name="etp", bufs=3)
        hp = tc.alloc_tile_pool(name="hp", bufs=2)
        op = tc.alloc_tile_pool(name="op", bufs=4)
        ps_work = tc.alloc_tile_pool(name="ps_work", bufs=1, space="PSUM")
        ps_tr = tc.alloc_tile_pool(name="ps_tr", bufs=1, space="PSUM")
        ps_sc = tc.alloc_tile_pool(name="ps_sc", bufs=2, space="PSUM")
        ps_h = tc.alloc_tile_pool(name="ps_h", bufs=1, space="PSUM")

        ident = consts.tile([128, 128], f32, tag="ident")
        make_identity(nc, ident[:, :])
        identb = consts.tile([128, 128], bf16, tag="identb")
        make_identity(nc, identb[:, :])

        def transpose4(dst_slices, src, chunks, tag="tr"):
            """PE-transpose `chunks` 128x128 bf16 blocks of `src`, packed 4
            per PSUM bank, one DVE copy per pack into dst_slices(kc0, n)."""
            for kc0 in range(0, chunks, 4):
                n = min(4, chunks - kc0)
                pt = ps_tr.tile([128, 4, 128], bf16, tag=tag)
                for j in range(n):
                    nc.tensor.transpose(
                        pt[:, j, :],
                        src[:, (kc0 + j) * 128:(kc0 + j + 1) * 128],
                        identb[:, :],
                    )
                nc.vector.tensor_copy(dst_slices(kc0, n), pt[:, 0:n, :])

        # --- weights: f32 load, DVE cast bf16, PE transpose to [d, w] ---
        wts = {}
        for name, wd in (("q", wq_d), ("k", wk_d), ("v", wv_d)):
            wf = xp.tile([128, D], f32, tag="xf")
            nc.scalar.dma_start(out=wf[:, :], in_=wd[:, :])
            wb = xbp.tile([128, D], bf16, tag="x")
            nc.vector.tensor_copy(wb[:, :], wf[:, :])
            wt = consts.tile([128, KC, 128], bf16, tag=f"wt_{name}")
            transpose4(lambda kc0, n, wt=wt: wt[:, kc0:kc0 + n, :], wb, KC)
            wts[name] = wt

        bias = {}
        for name, bd in (("q", bq_d), ("k", bk_d), ("v", bv_d)):
            bc = consts.tile([128, 1], f32, tag=f"b_{name}")
            nc.gpsimd.dma_start(
                out=bc[:, :], in_=bd.ap().rearrange("(p one) -> p one", one=1)
            )
            bias[name] = bc

        # --- mask -> em[t] = exp(1e4*m - 1e4), laid out [t_local, t_chunk] ---
        mb = consts.tile([128, 1], f32, tag="mbias")
        nc.vector.memset(mb[:, :], -10000.0)
        ems = []
        for b in range(B):
            msk = consts.tile([128, SC], f32, tag=f"mask{b}")
            nc.gpsimd.dma_start(
                out=msk[:, :], in_=m_d[b].rearrange("(c p) -> p c", p=128)
            )
            em = consts.tile([128, SC], f32, tag=f"em{b}")
            nc.scalar.activation(em[:, :], msk[:, :], EXP, scale=10000.0, bias=mb[:, :])
            ems.append(em)

        # xT tiles for both batches; b=1's odd s-chunks go through the DMA
        # XBAR (slow but free of PE cycles) hoisted here so they complete
        # underneath b=0's attention phase.
        xts = []
        for _b in range(B):
            xt_b = xtp.tile([128, KC, S], bf16, tag="xt")
            xts.append(xt_b)
        for sc in range(1, SC, 2):
            xf = xp.tile([128, D], f32, tag="x1f")
            nc.scalar.dma_start(out=xf[:, :], in_=x_d[1, sc * 128:(sc + 1) * 128, :])
            xb = xbp.tile([128, D], bf16, tag="x1b")
            nc.vector.tensor_copy(xb[:, :], xf[:, :])
            nc.sync.dma_start(
                out=xts[1][:, :, sc * 128:(sc + 1) * 128], in_=xb[:, :],
                transpose=True,
            )

        for b in range(B):
            # --- xT[d, s] bf16: f32 load, DVE cast, PE transpose ---
            xt = xts[b]
            pe_chunks = range(SC) if b == 0 else range(0, SC, 2)
            for sc in pe_chunks:
                xf = xp.tile([128, D], f32, tag="xf")
                nc.scalar.dma_start(out=xf[:, :], in_=x_d[b, sc * 128:(sc + 1) * 128, :])
                xb = xbp.tile([128, D], bf16, tag="x")
                nc.vector.tensor_copy(xb[:, :], xf[:, :])
                transpose4(
                    lambda kc0, n, sc=sc: xt[:, kc0:kc0 + n, sc * 128:(sc + 1) * 128],
                    xb, KC,
                )

            # --- projections: qT/kT/vT [w, s] = W.T-chunks @ xT ---
            qt = qkp.tile([128, S], bf16, tag="qt")
            kt = qkp.tile([128, S], bf16, tag="kt")
            vt = qkp.tile([128, S], bf16, tag="vt")
            for dst, wname in ((qt, "q"), (kt, "k"), (vt, "v")):
                wt = wts[wname]
                for sg in range(NSEG):
                    pp = ps_work.tile([128, SEG], f32, tag="work")
                    for kc in range(KC):
                        nc.tensor.matmul(
                            pp[:, :],
                            lhsT=wt[:, kc, :],
                            rhs=xt[:, kc, sg * SEG:(sg + 1) * SEG],
                            start=(kc == 0),
                            stop=(kc == KC - 1),
                        )
                    nc.vector.tensor_scalar_add(
                        dst[:, sg * SEG:(sg + 1) * SEG], pp[:, :], bias[wname][:, :]
                    )

            # --- v'' [t, (head, 65)]: PE transpose vt chunk, em scale, em col ---
            v2 = vp.tile([128, SC, HEADS, W + 1], bf16, tag="v2")
            for scc in range(SC):
                pv = ps_tr.tile([128, 4, 128], bf16, tag="tr")
                nc.tensor.transpose(
                    pv[:, 0, :], vt[:, scc * 128:(scc + 1) * 128], identb[:, :]
                )
                nc.vector.tensor_scalar(
                    out=v2[:, scc, :, 0:W],
                    in0=pv[:, 0, :].rearrange("p (h w) -> p h w", h=HEADS),
                    scalar1=ems[b][:, scc:scc + 1],
                    scalar2=None,
                    op0=mybir.AluOpType.mult,
                )
                for h in range(HEADS):
                    nc.vector.tensor_copy(
                        v2[:, scc, h, W:W + 1], ems[b][:, scc:scc + 1]
                    )

            # --- attention ---
            for h in range(HEADS):
                for blk in range(NBLK):
                    ph = ps_h.tile([W + 1, 2, SEG], f32, tag="ph")
                    for t in range(SC):
                        psc = ps_sc.tile([128, 2, SEG], f32, tag="sc")
                        for sg in range(2):
                            nc.tensor.matmul(
                                psc[:, sg, :],
                                lhsT=kt[h * W:(h + 1) * W, t * 128:(t + 1) * 128],
                                rhs=qt[h * W:(h + 1) * W,
                                       blk * SBLK + sg * SEG:blk * SBLK + (sg + 1) * SEG],
                                start=True,
                                stop=True,
                            )
                        et = etp.tile([128, 2, SEG], bf16, tag="et")
                        nc.scalar.activation(et[:, :, :], psc[:, :, :], EXP, scale=0.125)
                        for sg in range(2):
                            nc.tensor.matmul(
                                ph[:, sg, :],
                                lhsT=v2[:, t, h, :],
                                rhs=et[:, sg, :],
                                start=(t == 0),
                                stop=(t == SC - 1),
                            )
                    hsb = hp.tile([W + 1, SBLK], f32, tag="hsb")
                    nc.vector.tensor_copy(hsb[:, :], ph[:, :, :])
                    for ss in range(SBLK // 128):
                        pt = ps_work.tile([128, SEG], f32, tag="work")
                        nc.tensor.transpose(
                            pt[:, 0:W + 1],
                            hsb[:, ss * 128:(ss + 1) * 128],
                            ident[0:W + 1, 0:W + 1],
                        )
                        rec = op.tile([128, 1], f32, tag="rec")
                        nc.vector.reciprocal(rec[:, :], pt[:, W:W + 1])
                        ot = op.tile([128, W], f32, tag="ot")
                        nc.vector.tensor_scalar_mul(ot[:, :], pt[:, 0:W], rec[:, :])
                        s0 = blk * SBLK + ss * 128
                        nc.gpsimd.dma_start(
                            out=o_d[b, s0:s0 + 128, h * W:(h + 1) * W], in_=ot[:, :]
                        )

        for p in (ps_h, ps_sc, ps_tr, ps_work, op, hp, etp, vp, qkp, xtp, xbp, xp,
                  consts):
            p.release()

    nc.finalize()
    return nc


_NC = None


def _get_nc():
    global _NC
    if _NC is None:
        _NC = _build()
    return _NC


def _in_maps(inputs):
    x = np.ascontiguousarray(np.asarray(inputs["hidden_states"], dtype=np.float32))
    m = np.ascontiguousarray(np.asarray(inputs["attn_mask"], dtype=np.float32))
    maps = []
    for c in range(NCORES):
        sl = slice(c * WC, (c + 1) * WC)
        maps.append({
            "hidden_states": x,
            "attn_mask": m,
            "wq": np.ascontiguousarray(np.asarray(inputs["Wq"], dtype=np.float32)[sl]),
            "wk": np.ascontiguousarray(np.asarray(inputs["Wk"], dtype=np.float32)[sl]),
            "wv": np.ascontiguousarray(np.asarray(inputs["Wv"], dtype=np.float32)[sl]),
            "bq": np.ascontiguousarray(np.asarray(inputs["bq"], dtype=np.float32)[sl]),
            "bk": np.ascontiguousarray(np.asarray(inputs["bk"], dtype=np.float32)[sl]),
            "bv": np.ascontiguousarray(np.asarray(inputs["bv"], dtype=np.float32)[sl]),
        })
    return maps


def _run(inputs, trace=False):
    from concourse.bass_utils import run_bass_kernel_spmd

    nc = _get_nc()
    res = run_bass_kernel_spmd(
        nc, _in_maps(inputs), core_ids=list(range(NCORES)), trace=trace
    )
    out = np.concatenate([res.results[c]["out"] for c in range(NCORES)], axis=2)
    return np.ascontiguousarray(out, dtype=np.float32), res


def kernel(**inputs):
    out, _ = _run(inputs, trace=False)
    return out


# revision 23
# speedup vs baseline: 1.1354x; 1.1354x over previous
"""Multi-head attention (B=2, S=2048, D=1024, 16 heads x 64) on 8 TRN2 cores.

Sharding: tensor-parallel over heads. Core c owns heads {2c, 2c+1} =
rows [128c, 128c+128) of Wq/Wk/Wv, computes its (B, S, 128) slice of the
context, host concatenates along the feature axis. No collectives.

Per-core pipeline (matmul operands bf16, f32 PSUM accumulation):
  x, W: f32 HWDGE load -> DVE cast to bf16 -> PE transpose (1 cyc/row,
  4 chunks packed per PSUM bank) -> DVE copy to xT/wT.
  qT/kT/vT projections (+bias per-partition). v re-transposed to [t, w]
  on PE. mask -> em[t] = exp(-1e4*(1-mask[t])) folded into V rows
  (exact: exp(a+b) = exp(a)exp(b)); V carries an extra em column so the
  PV matmul also produces the softmax denominator Z.
  scoresT[t,s] = k[t].q[s] per 128-key chunk -> ACT exp (scale=1/8) from
  PSUM -> PV accumulate (65 x 512) -> PE transpose -> scale by 1/Z -> out
  (output DMA on GPSIMD/SWDGE to keep the HWDGE queues free).
"""

import sys

if "/opt/trn_rl_repo" not in sys.path:
    sys.path.insert(0, "/opt/trn_rl_repo")

import numpy as np

B = 2
S = 2048
D = 1024
NCORES = 8
WC = 128          # per-core projection width (2 heads x 64)
HEADS = 2         # heads per core
W = 64            # head dim
KC = D // 128     # contraction chunks (8)
SC = S // 128     # 128-row chunks of S (16)
SEG = 512         # matmul moving-dim segment
NSEG = S // SEG   # 4
SBLK = 1024       # attention s-block (2 segments)
NBLK = S // SBLK  # 2


def _build():
    import concourse.bass as bass
    import concourse.tile as tile
    from concourse import bacc, mybir
    from concourse.masks import make_identity

    f32 = mybir.dt.float32
    bf16 = mybir.dt.bfloat16
    EXP = mybir.ActivationFunctionType.Exp

    nc = bacc.Bacc("TRN2", target_bir_lowering=False, debug=False)

    x_d = nc.dram_tensor("hidden_states", [B, S, D], f32, kind="ExternalInput")
    m_d = nc.dram_tensor("attn_mask", [B, S], f32, kind="ExternalInput")
    wq_d = nc.dram_tensor("wq", [WC, D], f32, kind="ExternalInput")
    wk_d = nc.dram_tensor("wk", [WC, D], f32, kind="ExternalInput")
    wv_d = nc.dram_tensor("wv", [WC, D], f32, kind="ExternalInput")
    bq_d = nc.dram_tensor("bq", [WC], f32, kind="ExternalInput")
    bk_d = nc.dram_tensor("bk", [WC], f32, kind="ExternalInput")
    bv_d = nc.dram_tensor("bv", [WC], f32, kind="ExternalInput")
    o_d = nc.dram_tensor("out", [B, S, WC], f32, kind="ExternalOutput")

    with tile.TileContext(nc) as tc:
        consts = tc.alloc_tile_pool(name="consts", bufs=1)
        xp = tc.alloc_tile_pool(name="xp", bufs=4)
        xbp = tc.alloc_tile_pool(name="xbp", bufs=4)
        xtp = tc.alloc_tile_pool(name="xtp", bufs=2)
        qkp = tc.alloc_tile_pool(name="qkp", bufs=2)
        vp = tc.alloc_tile_pool(name="vp", bufs=2)
        etp = tc.alloc_tile_pool(name="etp", bufs=3)
        hp = tc.alloc_tile_pool(name="hp", bufs=2)
        op = tc.alloc_tile_pool(name="op", bufs=4)
        ps_work = tc.alloc_tile_pool(name="ps_work", bufs=1, space="PSUM")
        ps_tr = tc.alloc_tile_pool(name="ps_tr", bufs=1, space="PSUM")
        ps_sc = tc.alloc_tile_pool(name="ps_sc", bufs=2, space="PSUM")
        ps_h = tc.alloc_tile_pool(name="ps_h", bufs=1, space="PSUM")

        ident = consts.tile([128, 128], f32, tag="ident")
        make_identity(nc, ident[:, :])
        identb = consts.tile([128, 128], bf16, tag="identb")
        make_identity(nc, identb[:, :])

        def transpose4(dst_slices, src, chunks, tag="tr"):
            """PE-transpose `chunks` 128x128 bf16 blocks of `src`, packed 4
            per PSUM bank, one DVE copy per pack into dst_slices(kc0, n)."""
            for kc0 in range(0, chunks, 4):
                n = min(4, chunks - kc0)
                pt = ps_tr.tile([128, 4, 128], bf16, tag=tag)
                for j in range(n):
                    nc.tensor.transpose(
                        pt[:, j, :],
                        src[:, (kc0 + j) * 128:(kc0 + j + 1) * 128],
                        identb[:, :],
                    )
                nc.vector.tensor_copy(dst_slices(kc0, n), pt[:, 0:n, :])

        # --- weights: f32 load, DVE cast bf16, PE transpose to [d, w] ---
        wts = {}
        for name, wd in (("q", wq_d), ("k", wk_d), ("v", wv_d)):
            wf = xp.tile([128, D], f32, tag="xf")
            nc.scalar.dma_start(out=wf[:, :], in_=wd[:, :])
            wb = xbp.tile([128, D], bf16, tag="x")
            nc.vector.tensor_copy(wb[:, :], wf[:, :])
            wt = consts.tile([128, KC, 128], bf16, tag=f"wt_{name}")
            transpose4(lambda kc0, n, wt=wt: wt[:, kc0:kc0 + n, :], wb, KC)
            wts[name] = wt

        bias = {}
        for name, bd in (("q", bq_d), ("k", bk_d), ("v", bv_d)):
            bc = consts.tile([128, 1], f32, tag=f"b_{name}")
            nc.gpsimd.dma_start(
                out=bc[:, :], in_=bd.ap().rearrange("(p one) -> p one", one=1)
            )
            bias[name] = bc

        # --- mask -> em[t] = exp(1e4*m - 1e4), laid out [t_local, t_chunk] ---
        mb = consts.tile([128, 1], f32, tag="mbias")
        nc.vector.memset(mb[:, :], -10000.0)
        ems = []
        for b in range(B):
            msk = consts.tile([128, SC], f32, tag=f"mask{b}")
            nc.gpsimd.dma_start(
                out=msk[:, :], in_=m_d[b].rearrange("(c p) -> p c", p=128)
            )
            em = consts.tile([128, SC], f32, tag=f"em{b}")
            nc.scalar.activation(em[:, :], msk[:, :], EXP, scale=10000.0, bias=mb[:, :])
            ems.append(em)

        for b in range(B):
            # --- xT[d, s] bf16: f32 load, DVE cast, PE transpose ---
            xt = xtp.tile([128, KC, S], bf16, tag="xt")
            for sc in range(SC):
                xf = xp.tile([128, D], f32, tag="xf")
                nc.sync.dma_start(out=xf[:, :], in_=x_d[b, sc * 128:(sc + 1) * 128, :])
                xb = xbp.tile([128, D], bf16, tag="x")
                nc.vector.tensor_copy(xb[:, :], xf[:, :])
                transpose4(
                    lambda kc0, n, sc=sc: xt[:, kc0:kc0 + n, sc * 128:(sc + 1) * 128],
                    xb, KC,
                )

            # --- projections: qT/kT/vT [w, s] = W.T-chunks @ xT ---
            qt = qkp.tile([128, S], bf16, tag="qt")
            kt = qkp.tile([128, S], bf16, tag="kt")
            vt = qkp.tile([128, S], bf16, tag="vt")
            for dst, wname in ((qt, "q"), (kt, "k"), (vt, "v")):
                wt = wts[wname]
                for sg in range(NSEG):
                    pp = ps_work.tile([128, SEG], f32, tag="work")
                    for kc in range(KC):
                        nc.tensor.matmul(
                            pp[:, :],
                            lhsT=wt[:, kc, :],
                            rhs=xt[:, kc, sg * SEG:(sg + 1) * SEG],
                            start=(kc == 0),
                            stop=(kc == KC - 1),
                        )
                    nc.vector.tensor_scalar_add(
                        dst[:, sg * SEG:(sg + 1) * SEG], pp[:, :], bias[wname][:, :]
                    )

            # --- v'' [t, (head, 65)]: PE transpose vt chunk, em scale, em col ---
            v2 = vp.tile([128, SC, HEADS, W + 1], bf16, tag="v2")
            for scc in range(SC):
                pv = ps_tr.tile([128, 4, 128], bf16, tag="tr")
                nc.tensor.transpose(
                    pv[:, 0, :], vt[:, scc * 128:(scc + 1) * 128], identb[:, :]
                )
                nc.vector.tensor_scalar(
                    out=v2[:, scc, :, 0:W],
                    in0=pv[:, 0, :].rearrange("p (h w) -> p h w", h=HEADS),
                    scalar1=ems[b][:, scc:scc + 1],
                    scalar2=None,
                    op0=mybir.AluOpType.mult,
                )
                for h in range(HEADS):
                    nc.vector.tensor_copy(
                        v2[:, scc, h, W:W + 1], ems[b][:, scc:scc + 1]
                    )

            # --- attention ---
            for h in range(HEADS):
                for blk in range(NBLK):
                    ph = ps_h.tile([W + 1, 2, SEG], f32, tag="ph")
                    for t in range(SC):
                        psc = ps_sc.tile([128, 2, SEG], f32, tag="sc")
                        for sg in range(2):
                            nc.tensor.matmul(
                                psc[:, sg, :],
                                lhsT=kt[h * W:(h + 1) * W, t * 128:(t + 1) * 128],
                                rhs=qt[h * W:(h + 1) * W,
                                       blk * SBLK + sg * SEG:blk * SBLK + (sg + 1) * SEG],
                                start=True,
                                stop=True,
                            )
                        et = etp.tile([128, 2, SEG], bf16, tag="et")
                        nc.scalar.activation(et[:, :, :], psc[:, :, :], EXP, scale=0.125)
                        for sg in range(2):
                            nc.tensor.matmul(
                                ph[:, sg, :],
                                lhsT=v2[:, t, h, :],
                                rhs=et[:, sg, :],
                                start=(t == 0),
                                stop=(t == SC - 1),
                            )
                    hsb = hp.tile([W + 1, SBLK], f32, tag="hsb")
                    nc.vector.tensor_copy(hsb[:, :], ph[:, :, :])
                    for ss in range(SBLK // 128):
                        pt = ps_work.tile([128, SEG], f32, tag="work")
                        nc.tensor.transpose(
                            pt[:, 0:W + 1],
                            hsb[:, ss * 128:(ss + 1) * 128],
                            ident[0:W + 1, 0:W + 1],
                        )
                        rec = op.tile([128, 1], f32, tag="rec")
                        nc.vector.reciprocal(rec[:, :], pt[:, W:W + 1])
                        ot = op.tile([128, W], f32, tag="ot")
                        nc.vector.tensor_scalar_mul(ot[:, :], pt[:, 0:W], rec[:, :])
                        s0 = blk * SBLK + ss * 128
                        nc.gpsimd.dma_start(
                            out=o_d[b, s0:s0 + 128, h * W:(h + 1) * W], in_=ot[:, :]
                        )

        for p in (ps_h, ps_sc, ps_tr, ps_work, op, hp, etp, vp, qkp, xtp, xbp, xp,
                  consts):
            p.release()

    nc.finalize()
    return nc


_NC = None


def _get_nc():
    global _NC
    if _NC is None:
        _NC = _build()
    return _NC


def _in_maps(inputs):
    x = np.ascontiguousarray(np.asarray(inputs["hidden_states"], dtype=np.float32))
    m = np.ascontiguousarray(np.asarray(inputs["attn_mask"], dtype=np.float32))
    maps = []
    for c in range(NCORES):
        sl = slice(c * WC, (c + 1) * WC)
        maps.append({
            "hidden_states": x,
            "attn_mask": m,
            "wq": np.ascontiguousarray(np.asarray(inputs["Wq"], dtype=np.float32)[sl]),
            "wk": np.ascontiguousarray(np.asarray(inputs["Wk"], dtype=np.float32)[sl]),
            "wv": np.ascontiguousarray(np.asarray(inputs["Wv"], dtype=np.float32)[sl]),
            "bq": np.ascontiguousarray(np.asarray(inputs["bq"], dtype=np.float32)[sl]),
            "bk": np.ascontiguousarray(np.asarray(inputs["bk"], dtype=np.float32)[sl]),
            "bv": np.ascontiguousarray(np.asarray(inputs["bv"], dtype=np.float32)[sl]),
        })
    return maps


def _run(inputs, trace=False):
    from concourse.bass_utils import run_bass_kernel_spmd

    nc = _get_nc()
    res = run_bass_kernel_spmd(
        nc, _in_maps(inputs), core_ids=list(range(NCORES)), trace=trace
    )
    out = np.concatenate([res.results[c]["out"] for c in range(NCORES)], axis=2)
    return np.ascontiguousarray(out, dtype=np.float32), res


def kernel(**inputs):
    out, _ = _run(inputs, trace=False)
    return out


# revision 24
# speedup vs baseline: 1.2647x; 1.1139x over previous
"""Multi-head attention (B=2, S=2048, D=1024, 16 heads x 64) on 8 TRN2 cores.

Sharding: tensor-parallel over heads. Core c owns heads {2c, 2c+1} =
rows [128c, 128c+128) of Wq/Wk/Wv, computes its (B, S, 128) slice of the
context, host concatenates along the feature axis. No collectives.

Per-core pipeline (matmul operands bf16, f32 PSUM accumulation):
  x, W: f32 HWDGE load -> DVE cast to bf16 -> PE transpose (1 cyc/row,
  4 chunks packed per PSUM bank) -> DVE copy to xT/wT.
  qT/kT/vT projections (+bias per-partition). v re-transposed to [t, w]
  on PE. mask -> em[t] = exp(-1e4*(1-mask[t])) folded into V rows
  (exact: exp(a+b) = exp(a)exp(b)); V carries an extra em column so the
  PV matmul also produces the softmax denominator Z.
  scoresT[t,s] = k[t].q[s] per 128-key chunk -> ACT exp (scale=1/8) from
  PSUM -> PV accumulate (65 x 512) -> PE transpose -> scale by 1/Z -> out
  (output DMA on GPSIMD/SWDGE to keep the HWDGE queues free).
"""

import sys

if "/opt/trn_rl_repo" not in sys.path:
    sys.path.insert(0, "/opt/trn_rl_repo")

import numpy as np

B = 2
S = 2048
D = 1024
NCORES = 8
WC = 128          # per-core projection width (2 heads x 64)
HEADS = 2         # heads per core
W = 64            # head dim
KC = D // 128     # contraction chunks (8)
SC = S // 128     # 128-row chunks of S (16)
SEG = 512         # matmul moving-dim segment
NSEG = S // SEG   # 4
SBLK = 512        # attention s-block
NBLK = S // SBLK  # 4


def _build():
    import concourse.bass as bass
    import concourse.tile as tile
    from concourse import bacc, mybir
    from concourse.masks import make_identity

    f32 = mybir.dt.float32
    bf16 = mybir.dt.bfloat16
    EXP = mybir.ActivationFunctionType.Exp

    nc = bacc.Bacc("TRN2", target_bir_lowering=False, debug=False)

    x_d = nc.dram_tensor("hidden_states", [B, S, D], f32, kind="ExternalInput")
    m_d = nc.dram_tensor("attn_mask", [B, S], f32, kind="ExternalInput")
    wq_d = nc.dram_tensor("wq", [WC, D], f32, kind="ExternalInput")
    wk_d = nc.dram_tensor("wk", [WC, D], f32, kind="ExternalInput")
    wv_d = nc.dram_tensor("wv", [WC, D], f32, kind="ExternalInput")
    bq_d = nc.dram_tensor("bq", [WC], f32, kind="ExternalInput")
    bk_d = nc.dram_tensor("bk", [WC], f32, kind="ExternalInput")
    bv_d = nc.dram_tensor("bv", [WC], f32, kind="ExternalInput")
    o_d = nc.dram_tensor("out", [B, S, WC], f32, kind="ExternalOutput")

    with tile.TileContext(nc) as tc:
        consts = tc.alloc_tile_pool(name="consts", bufs=1)
        xp = tc.alloc_tile_pool(name="xp", bufs=4)
        xbp = tc.alloc_tile_pool(name="xbp", bufs=4)
        xtp = tc.alloc_tile_pool(name="xtp", bufs=2)
        qkp = tc.alloc_tile_pool(name="qkp", bufs=2)
        vp = tc.alloc_tile_pool(name="vp", bufs=2)
        etp = tc.alloc_tile_pool(name="etp", bufs=4)
        hp = tc.alloc_tile_pool(name="hp", bufs=2)
        op = tc.alloc_tile_pool(name="op", bufs=4)
        ps_work = tc.alloc_tile_pool(name="ps_work", bufs=1, space="PSUM")
        ps_tr = tc.alloc_tile_pool(name="ps_tr", bufs=2, space="PSUM")
        ps_sc = tc.alloc_tile_pool(name="ps_sc", bufs=2, space="PSUM")
        ps_h = tc.alloc_tile_pool(name="ps_h", bufs=1, space="PSUM")

        ident = consts.tile([128, 128], f32, tag="ident")
        make_identity(nc, ident[:, :])
        identb = consts.tile([128, 128], bf16, tag="identb")
        make_identity(nc, identb[:, :])

        def transpose4(dst_slices, src, chunks, tag="tr"):
            """PE-transpose `chunks` 128x128 bf16 blocks of `src`, packed 4
            per PSUM bank, one DVE copy per pack into dst_slices(kc0, n)."""
            for kc0 in range(0, chunks, 4):
                n = min(4, chunks - kc0)
                pt = ps_tr.tile([128, 4, 128], bf16, tag=tag)
                for j in range(n):
                    nc.tensor.transpose(
                        pt[:, j, :],
                        src[:, (kc0 + j) * 128:(kc0 + j + 1) * 128],
                        identb[:, :],
                    )
                nc.vector.tensor_copy(dst_slices(kc0, n), pt[:, 0:n, :])

        # --- weights: f32 load, DVE cast bf16, PE transpose to [d, w] ---
        wts = {}
        for name, wd in (("q", wq_d), ("k", wk_d), ("v", wv_d)):
            wf = xp.tile([128, D], f32, tag="xf")
            nc.scalar.dma_start(out=wf[:, :], in_=wd[:, :])
            wb = xbp.tile([128, D], bf16, tag="x")
            nc.vector.tensor_copy(wb[:, :], wf[:, :])
            wt = consts.tile([128, KC, 128], bf16, tag=f"wt_{name}")
            transpose4(lambda kc0, n, wt=wt: wt[:, kc0:kc0 + n, :], wb, KC)
            wts[name] = wt

        bias = {}
        for name, bd in (("q", bq_d), ("k", bk_d), ("v", bv_d)):
            bc = consts.tile([128, 1], f32, tag=f"b_{name}")
            nc.gpsimd.dma_start(
                out=bc[:, :], in_=bd.ap().rearrange("(p one) -> p one", one=1)
            )
            bias[name] = bc

        # --- mask -> em[t] = exp(1e4*m - 1e4), laid out [t_local, t_chunk] ---
        mb = consts.tile([128, 1], f32, tag="mbias")
        nc.vector.memset(mb[:, :], -10000.0)
        ems = []
        for b in range(B):
            msk = consts.tile([128, SC], f32, tag=f"mask{b}")
            nc.gpsimd.dma_start(
                out=msk[:, :], in_=m_d[b].rearrange("(c p) -> p c", p=128)
            )
            em = consts.tile([128, SC], f32, tag=f"em{b}")
            nc.scalar.activation(em[:, :], msk[:, :], EXP, scale=10000.0, bias=mb[:, :])
            ems.append(em)

        for b in range(B):
            # --- xT[d, s] bf16: f32 load, DVE cast, PE transpose ---
            xt = xtp.tile([128, KC, S], bf16, tag="xt")
            for sc in range(SC):
                xf = xp.tile([128, D], f32, tag="xf")
                nc.sync.dma_start(out=xf[:, :], in_=x_d[b, sc * 128:(sc + 1) * 128, :])
                xb = xbp.tile([128, D], bf16, tag="x")
                nc.vector.tensor_copy(xb[:, :], xf[:, :])
                transpose4(
                    lambda kc0, n, sc=sc: xt[:, kc0:kc0 + n, sc * 128:(sc + 1) * 128],
                    xb, KC,
                )

            # --- projections: qT/kT/vT [w, s] = W.T-chunks @ xT ---
            qt = qkp.tile([128, S], bf16, tag="qt")
            kt = qkp.tile([128, S], bf16, tag="kt")
            vt = qkp.tile([128, S], bf16, tag="vt")
            for dst, wname in ((qt, "q"), (kt, "k"), (vt, "v")):
                wt = wts[wname]
                for sg in range(NSEG):
                    pp = ps_work.tile([128, SEG], f32, tag="work")
                    for kc in range(KC):
                        nc.tensor.matmul(
                            pp[:, :],
                            lhsT=wt[:, kc, :],
                            rhs=xt[:, kc, sg * SEG:(sg + 1) * SEG],
                            start=(kc == 0),
                            stop=(kc == KC - 1),
                        )
                    nc.vector.tensor_scalar_add(
                        dst[:, sg * SEG:(sg + 1) * SEG], pp[:, :], bias[wname][:, :]
                    )

            # --- v'' [t, (head, 65)]: PE transpose vt chunk, em scale, em col ---
            v2 = vp.tile([128, SC, HEADS, W + 1], bf16, tag="v2")
            for scc in range(SC):
                pv = ps_tr.tile([128, 4, 128], bf16, tag="tr")
                nc.tensor.transpose(
                    pv[:, 0, :], vt[:, scc * 128:(scc + 1) * 128], identb[:, :]
                )
                nc.vector.tensor_scalar(
                    out=v2[:, scc, :, 0:W],
                    in0=pv[:, 0, :].rearrange("p (h w) -> p h w", h=HEADS),
                    scalar1=ems[b][:, scc:scc + 1],
                    scalar2=None,
                    op0=mybir.AluOpType.mult,
                )
                for h in range(HEADS):
                    nc.vector.tensor_copy(
                        v2[:, scc, h, W:W + 1], ems[b][:, scc:scc + 1]
                    )

            # --- attention: s-block 512, two t-chunks packed per PSUM tile ---
            for h in range(HEADS):
                for blk in range(NBLK):
                    ph = ps_h.tile([W + 1, SEG], f32, tag="ph")
                    for tp in range(0, SC, 2):
                        psc = ps_sc.tile([128, 2, SEG], f32, tag="sc")
                        for j in range(2):
                            nc.tensor.matmul(
                                psc[:, j, :],
                                lhsT=kt[h * W:(h + 1) * W,
                                        (tp + j) * 128:(tp + j + 1) * 128],
                                rhs=qt[h * W:(h + 1) * W,
                                       blk * SBLK:(blk + 1) * SBLK],
                                start=True,
                                stop=True,
                            )
                        et = etp.tile([128, 2, SEG], bf16, tag="et")
                        nc.scalar.activation(et[:, :, :], psc[:, :, :], EXP, scale=0.125)
                        for j in range(2):
                            nc.tensor.matmul(
                                ph[:, :],
                                lhsT=v2[:, tp + j, h, :],
                                rhs=et[:, j, :],
                                start=(tp == 0 and j == 0),
                                stop=(tp == SC - 2 and j == 1),
                            )
                    hsb = hp.tile([W + 1, SBLK], f32, tag="hsb")
                    nc.vector.tensor_copy(hsb[:, :], ph[:, :])
                    for ss in range(SBLK // 128):
                        pt = ps_work.tile([128, SEG], f32, tag="work")
                        nc.tensor.transpose(
                            pt[:, 0:W + 1],
                            hsb[:, ss * 128:(ss + 1) * 128],
                            ident[0:W + 1, 0:W + 1],
                        )
                        rec = op.tile([128, 1], f32, tag="rec")
                        nc.vector.reciprocal(rec[:, :], pt[:, W:W + 1])
                        ot = op.tile([128, W], f32, tag="ot")
                        nc.vector.tensor_scalar_mul(ot[:, :], pt[:, 0:W], rec[:, :])
                        s0 = blk * SBLK + ss * 128
                        nc.gpsimd.dma_start(
                            out=o_d[b, s0:s0 + 128, h * W:(h + 1) * W], in_=ot[:, :]
                        )

        for p in (ps_h, ps_sc, ps_tr, ps_work, op, hp, etp, vp, qkp, xtp, xbp, xp,
                  consts):
            p.release()

    nc.finalize()
    return nc


_NC = None


def _get_nc():
    global _NC
    if _NC is None:
        _NC = _build()
    return _NC


def _in_maps(inputs):
    x = np.ascontiguousarray(np.asarray(inputs["hidden_states"], dtype=np.float32))
    m = np.ascontiguousarray(np.asarray(inputs["attn_mask"], dtype=np.float32))
    maps = []
    for c in range(NCORES):
        sl = slice(c * WC, (c + 1) * WC)
        maps.append({
            "hidden_states": x,
            "attn_mask": m,
            "wq": np.ascontiguousarray(np.asarray(inputs["Wq"], dtype=np.float32)[sl]),
            "wk": np.ascontiguousarray(np.asarray(inputs["Wk"], dtype=np.float32)[sl]),
            "wv": np.ascontiguousarray(np.asarray(inputs["Wv"], dtype=np.float32)[sl]),
            "bq": np.ascontiguousarray(np.asarray(inputs["bq"], dtype=np.float32)[sl]),
            "bk": np.ascontiguousarray(np.asarray(inputs["bk"], dtype=np.float32)[sl]),
            "bv": np.ascontiguousarray(np.asarray(inputs["bv"], dtype=np.float32)[sl]),
        })
    return maps


def _run(inputs, trace=False):
    from concourse.bass_utils import run_bass_kernel_spmd

    nc = _get_nc()
    res = run_bass_kernel_spmd(
        nc, _in_maps(inputs), core_ids=list(range(NCORES)), trace=trace
    )
    out = np.concatenate([res.results[c]["out"] for c in range(NCORES)], axis=2)
    return np.ascontiguousarray(out, dtype=np.float32), res


def kernel(**inputs):
    out, _ = _run(inputs, trace=False)
    return out
